# revision 5
# baseline (speedup 1.0000x reference)
"""Trainium2 Bass kernel for BiLinearLayerV2.

  biLinear[b,f,d] = sum_e feature[b,f,e] * weight[f,e,d]
  out[b,f,g,d]    = biLinear[b,f,d] * feature[b,g,d] * weightLeft[f,g]

Shapes: feature [512,64,32] f32, weight [64,32,32], weightLeft [64,64]
Output: [512,64,64,32] f32 (256 MB) -> heavily store-bound.

Strategy: data-parallel over batch (64 per core x 8 cores). Per core:
  phase 1 (PE): C[b,f,d] via 64 small matmuls (contract e'), ACT-copied to
    SBUF as Ct_all[32*(f%4)+d, (f//4)*64+b].
  phase 2 (PE): per f one K=32 matmul Q_f[b,(g,d)] = C[b,f,d]*W[f,g] using a
    host-precomputed block-diagonal rhs Wdiag[f][d',(g,d)] = W[f,g]*delta(d,d').
    4 f's run concurrently in disjoint PE row/col groups (tile_position).
  DVE: one tensor_mul pass out = Q * feature (PSUM->SBUF).
  DMA: 1 MB stores, 8 KB contiguous per partition.
"""

import sys

if "/opt/trn_rl_repo" not in sys.path:
    sys.path.insert(0, "/opt/trn_rl_repo")

import numpy as np

B, F, E = 512, 64, 32
NCORES = 8
BLOC = B // NCORES  # 64
GD = F * E  # 2048

_cached = None


def _build_nc():
    from contextlib import ExitStack

    import concourse.bass as bass
    import concourse.tile as tile
    from concourse import bacc, mybir

    f32 = mybir.dt.float32
    nc = bacc.Bacc("TRN2", target_bir_lowering=False, debug=False)

    featd = nc.dram_tensor("featd", (2 * BLOC, GD), f32, kind="ExternalInput").ap()
    featT = nc.dram_tensor("featT", (E, F * BLOC), f32, kind="ExternalInput").ap()
    wT = nc.dram_tensor("wT", (E, F * E), f32, kind="ExternalInput").ap()
    wdiag = nc.dram_tensor("wdiag", (16, 128, GD), f32, kind="ExternalInput").ap()
    out = nc.dram_tensor("out", (BLOC, F, GD), f32, kind="ExternalOutput").ap()

    with tile.TileContext(nc) as tc, ExitStack() as ctx:
        consts = ctx.enter_context(tc.tile_pool(name="consts", bufs=1))
        featd_t = consts.tile([128, GD], f32)
        nc.scalar.dma_start(featd_t[:], featd)
        featT_t = consts.tile([E, F * BLOC], f32)
        nc.scalar.dma_start(featT_t[:], featT)
        wT_t = consts.tile([E, F * E], f32)
        nc.scalar.dma_start(wT_t[:], wT)
        ct_t = consts.tile([128, 16 * BLOC], f32)

        # Phase 1: C[b,f,d]. Per f: psum[d,b] = wT[:,f]^T(e',d) @ featT[:,f](e',b)
        with tc.tile_pool(name="psc", bufs=2, space=bass.MemorySpace.PSUM) as psc:
            for j4 in range(16):
                pc = psc.tile([128, BLOC], f32)
                for r in range(4):
                    f = 4 * j4 + r
                    nc.tensor.matmul(
                        pc[32 * r : 32 * r + 32, :],
                        wT_t[:, f * E : (f + 1) * E],
                        featT_t[:, f * BLOC : (f + 1) * BLOC],
                        start=True,
                        stop=True,
                        tile_position=(0, 32 * r),
                    )
                nc.scalar.copy(ct_t[:, j4 * BLOC : (j4 + 1) * BLOC], pc[:])

        # Phase 2: Q = C*W via block-diag rhs; out = Q * feature; store.
        with (
            tc.tile_pool(name="wd", bufs=2) as wdp,
            tc.tile_pool(name="psq", bufs=2, space=bass.MemorySpace.PSUM) as psq,
            tc.tile_pool(name="ot", bufs=3) as otp,
        ):
            for j4 in range(16):
                wdt = wdp.tile([128, GD], f32)
                nc.scalar.dma_start(wdt[:], wdiag[j4])
                for s in range(2):
                    pq = psq.tile([128, GD], f32)
                    for half in range(2):
                        r = 2 * s + half
                        lhsT = ct_t[32 * r : 32 * r + 32, j4 * BLOC : (j4 + 1) * BLOC]
                        for n in range(4):
                            nc.tensor.matmul(
                                pq[64 * half : 64 * half + 64, 512 * n : 512 * (n + 1)],
                                lhsT,
                                wdt[32 * r : 32 * r + 32, 512 * n : 512 * (n + 1)],
                                start=True,
                                stop=True,
                                tile_position=(32 * r, 64 * half),
                            )
                    ot = otp.tile([128, GD], f32)
                    nc.vector.tensor_mul(ot[:], pq[:], featd_t[:])
                    f0 = 4 * j4 + 2 * s
                    nc.sync.dma_start(out[:, f0, :], ot[0:64, :])
                    nc.sync.dma_start(out[:, f0 + 1, :], ot[64:128, :])

    nc.compile()
    return nc


def _get_nc():
    global _cached
    if _cached is None:
        _cached = _build_nc()
    return _cached


def _host_inputs(feature, weight, weightLeft):
    """Per-core input maps. Host work is layout prep only (transpose/replicate)."""
    feature = np.ascontiguousarray(feature, dtype=np.float32)
    weight = np.ascontiguousarray(weight, dtype=np.float32)
    weightLeft = np.ascontiguousarray(weightLeft, dtype=np.float32)

    wT = np.ascontiguousarray(weight.transpose(1, 0, 2)).reshape(E, F * E)
    # wdiag[f][d',(g,d)] = W[f,g]*delta(d,d'), grouped 4 f's per 128 partitions
    eye = np.eye(E, dtype=np.float32)
    wd = weightLeft[:, None, :, None] * eye[:, None, :]  # [F, E(d'), F(g), E(d)]
    wdiag = np.ascontiguousarray(wd).reshape(16, 4 * E, F * E)

    in_maps = []
    for c in range(NCORES):
        fc = feature[c * BLOC : (c + 1) * BLOC]  # [64, 64, 32]
        featd = np.ascontiguousarray(
            np.tile(fc.reshape(BLOC, GD), (2, 1))
        )  # [128, 2048]
        featT = np.ascontiguousarray(fc.transpose(2, 1, 0)).reshape(E, F * BLOC)
        in_maps.append({"featd": featd, "featT": featT, "wT": wT, "wdiag": wdiag})
    return in_maps


def _run(in_maps, trace=False, tmpdir=None):
    from concourse.bass_utils import run_bass_kernel_spmd

    nc = _get_nc()
    return run_bass_kernel_spmd(
        nc, in_maps, core_ids=list(range(NCORES)), trace=trace, tmpdir=tmpdir
    )


def kernel(feature, weight, weightLeft):
    in_maps = _host_inputs(feature, weight, weightLeft)
    res = _run(in_maps, trace=False)
    out = np.concatenate(
        [r["out"].reshape(BLOC, F, F, E) for r in res.results], axis=0
    )
    return out
